# revision 45
# baseline (speedup 1.0000x reference)
"""Multi-head attention (B=2, S=2048, D=1024, H=16) on 8 TRN2 NeuronCores.

Sharding: core c handles batch c//4 and head-group c%4 (4 heads each).
Host pre-transposes inputs/weights to d-major bf16; each core computes its
4 heads' projections, causal attention, and a partial (row-parallel) dense
output [S, D] (bf16) which the host sums across the 4 cores of each batch.

Attention math: scores are computed transposed ([k, q] layout, q on the
free dim) so no on-chip transposes are ever needed.  Masking is applied as
a multiplicative factor F = exp(-1e9*m/8) on the exp'd scores (exact for
0/1 masks, correct in general); fully-masked 128x512 tiles are skipped at
emit time based on the actual mask contents.  Softmax row sums come for
free from a ones-column appended to the V tiles in the AV matmul; the
reciprocal is broadcast across partitions via a small DRAM bounce.

Pipelining: emission is generator-driven.  Attention score groups are
emitted with a one-group skew (scores of group g+1 precede the AV matmuls
of group g) so the PE never head-of-line blocks on the ScalarE exp, and
projection pieces for chunk sc+1 plus dense pieces for chunk sc-1 are
interleaved between attention groups of chunk sc, giving the in-order PE
queue independent work wherever the softmax chain would stall it.  PSUM:
score groups alternate between a 3-bank and a 2-bank single-buffer pool
(depth-2 pipeline), 1 bank accumulates AV, and 2 banks rotate between
projection and dense matmuls.
"""

import numpy as np
import ml_dtypes
from contextlib import ExitStack

import concourse.bass as bass
import concourse.tile as tile
from concourse import bacc, mybir
from concourse.bass_utils import run_bass_kernel_spmd

BF16 = mybir.dt.bfloat16
F32 = mybir.dt.float32
NPBF16 = ml_dtypes.bfloat16

D_MODEL = 1024
NH = 16
DEPTH = 64
B = 2
S = 2048
N_CORES = 8
GROUPS = 4              # head-groups (tensor parallel dimension)
HPG = NH // GROUPS      # 4 heads per core
OG = HPG * DEPTH        # 256 projection output cols per core
QC = 512                # q chunk (matmul free dim)
NQC = S // QC           # 4
KT = 128                # k tile (psum partition dim)
NKT = S // KT           # 16
DK = D_MODEL // 128     # 8 contraction tiles of 128
SC = 512                # projection s chunk
NSC = S // SC           # 4

TRACE = False
TRACE_KW = {}
LAST_RESULT = None
_CACHE = {}


def _make_groups(tiles, want=2):
    """Split a chain's k-tiles into groups of `want` (2 or 3 — paired
    chains use a dedicated psum pool each, sized accordingly, so their
    interleaved groups never contend for the same banks).  Returns
    [([entries], lo_min)]; lo_min trims the exp to the live columns."""
    groups = []
    rem = tiles
    while rem:
        n = min(want, len(rem))
        groups.append((rem[:n], min(e[1] for e in rem[:n])))
        rem = rem[n:]
    return groups


def _build(ktiles, n_uniq, zero_bias):
    nc = bacc.Bacc(
        "TRN2", target_bir_lowering=False, debug=False, num_devices=N_CORES
    )
    # inputs pre-split into contiguous S-quarters for fat DMA descriptors
    xq = nc.dram_tensor("xq", [NSC, 128, DK, SC], BF16, kind="ExternalInput").ap()
    xk = nc.dram_tensor("xk", [NSC, 128, DK, SC], BF16, kind="ExternalInput").ap()
    xv = nc.dram_tensor("xv", [NSC, 128, DK, SC], BF16, kind="ExternalInput").ap()
    wq = nc.dram_tensor("wq", [128, DK, OG], BF16, kind="ExternalInput").ap()
    wk = nc.dram_tensor("wk", [128, DK, OG], BF16, kind="ExternalInput").ap()
    wv = nc.dram_tensor("wv", [128, DK, OG], BF16, kind="ExternalInput").ap()
    wd = nc.dram_tensor("wd", [128, 2, D_MODEL], BF16, kind="ExternalInput").ap()
    qb = nc.dram_tensor("qb", [128, 2], F32, kind="ExternalInput").ap()
    kb = nc.dram_tensor("kb", [128, 2], F32, kind="ExternalInput").ap()
    mk = nc.dram_tensor("mk", [128, n_uniq, KT], BF16, kind="ExternalInput").ap()
    outp = nc.dram_tensor("outp", [S, D_MODEL], BF16, kind="ExternalOutput").ap()

    Exp = mybir.ActivationFunctionType.Exp

    with tile.TileContext(nc) as tc, ExitStack() as ctx:
        singles = ctx.enter_context(tc.tile_pool(name="singles", bufs=1))
        exps = ctx.enter_context(tc.tile_pool(name="exps", bufs=8))
        small = ctx.enter_context(tc.tile_pool(name="small", bufs=3))
        bcastp = ctx.enter_context(tc.tile_pool(name="bcastp", bufs=4))
        dram = ctx.enter_context(tc.tile_pool(name="dram", bufs=3, space="DRAM"))
        ost = ctx.enter_context(tc.tile_pool(name="ost", bufs=3))
        xin = ctx.enter_context(tc.tile_pool(name="xin", bufs=6))
        # PSUM: 3+2 banks of alternating score groups (depth-2 pipeline),
        # 1 bank AV accumulator, 2 banks rotating proj/dense matmuls
        psc3 = ctx.enter_context(tc.tile_pool(name="psc3", bufs=1, space="PSUM"))
        psc2 = ctx.enter_context(tc.tile_pool(name="psc2", bufs=1, space="PSUM"))
        pav = ctx.enter_context(tc.tile_pool(name="pav", bufs=2, space="PSUM"))
        pmix = ctx.enter_context(tc.tile_pool(name="pmix", bufs=1, space="PSUM"))

        wq_sb = singles.tile([128, DK, OG], BF16)
        wk_sb = singles.tile([128, DK, OG], BF16)
        wv_sb = singles.tile([128, DK, OG], BF16)
        mk_sb = singles.tile([128, n_uniq, KT], BF16)
        qb_sb = singles.tile([128, 2], F32)
        kb_sb = singles.tile([128, 2], F32)
        wd_sb = singles.tile([128, 2, D_MODEL], BF16)

        # per-head layouts, zero-padded to K=128 so the scores matmuls keep
        # the PE's HAM activity monitor warm (K=64 streams never unthrottle).
        # head h occupies d-rows [(h%2)*64, (h%2)*64+64); the rest are zeros.
        qt = singles.tile([128, HPG, S], BF16)
        kt_ = singles.tile([128, HPG, S], BF16)
        # [p=k%128, ktile, head, 64 ones cols + 64 d cols]: the replicated
        # ones columns make the AV matmul emit the softmax denominator on
        # psum partitions 0-63 (64 identical lanes) and av^T on 64-127, so
        # the reciprocal runs lane-parallel straight off psum at base
        # partition 0 — no cross-partition bounce of the denominator
        vh1 = singles.tile([128, NKT, HPG, 128], BF16)
        avf = singles.tile([128, 2, S], F32)    # unnormalized av^T
        avb = singles.tile([128, 2, S], BF16)   # normalized av^T

        # startup loads first — before the gpsimd memsets, and spread over
        # both HWDGE rings (sync + scalar) so they fill in parallel; the
        # first projection matmul can start ~4.5us in
        x0 = {}
        for key, ap_ in (("q", xq), ("k", xk), ("v", xv)):
            t_ = xin.tile([128, DK, SC], BF16, tag="xin")
            x0[key] = t_
        # first-chunk loads split into dk-halves so the first matmuls can
        # start as soon as the leading half lands
        nc.sync.dma_start(x0["q"][:, 0 : DK // 2, :], xq[0, :, 0 : DK // 2, :])
        nc.sync.dma_start(wq_sb[:], wq)
        nc.sync.dma_start(x0["q"][:, DK // 2 :, :], xq[0, :, DK // 2 :, :])
        nc.scalar.dma_start(x0["k"][:, 0 : DK // 2, :], xk[0, :, 0 : DK // 2, :])
        nc.scalar.dma_start(wk_sb[:], wk)
        nc.scalar.dma_start(x0["k"][:, DK // 2 :, :], xk[0, :, DK // 2 :, :])
        nc.sync.dma_start(mk_sb[:], mk)
        nc.sync.dma_start(wv_sb[:], wv)
        nc.scalar.dma_start(x0["v"][:], xv[0])
        if not zero_bias:
            nc.sync.dma_start(qb_sb[:], qb)
            nc.sync.dma_start(kb_sb[:], kb)

        # warm-up matmuls on a memset scratch tile: they run during the
        # startup preamble + input-DMA wait and hold the PE's HAM
        # clock-gate open, so the first projection matmuls run at 2.4 GHz
        # instead of the cold 1.2 GHz
        warm = singles.tile([128, SC], BF16)
        nc.gpsimd.memset(warm[:], 1.0)
        for _wi in range(36):
            ps_w = pmix.tile([128, SC], F32, tag="mm")
            nc.tensor.matmul(
                ps_w[:], lhsT=warm[:, 0:128], rhs=warm[:], start=True, stop=True
            )

        nc.gpsimd.memset(vh1[:, :, :, 0:64], 1.0)
        # zero the padding halves of qt/kt (even heads: rows 64:128 unused,
        # odd heads: rows 0:64 unused)
        qtv = qt.rearrange("p (a b) s -> p a b s", b=2)
        ktv = kt_.rearrange("p (a b) s -> p a b s", b=2)
        nc.gpsimd.memset(qtv[64:128, :, 0, :], 0.0)
        nc.gpsimd.memset(qtv[0:64, :, 1, :], 0.0)
        nc.gpsimd.memset(ktv[64:128, :, 0, :], 0.0)
        nc.gpsimd.memset(ktv[0:64, :, 1, :], 0.0)

        def proj_load(sc):
            """Issue the q/k/v chunk loads for s-chunk sc."""
            x_sbs = {}
            for key, ap_ in (("q", xq), ("k", xk), ("v", xv)):
                t_ = xin.tile([128, DK, SC], BF16, tag="xin")
                nc.sync.dma_start(t_[:], ap_[sc])
                x_sbs[key] = t_
            return x_sbs

        def proj_pieces(sc, x_sbs, which="qkv", ocs=(0, 1)):
            """Generator: project q/k/v for s-chunk sc, yielding between
            independent pieces so attention groups can interleave.  `ocs`
            picks the output-column halves: oc=0 feeds heads 0/1, oc=1
            heads 2/3 — the first attention pair only needs oc=0."""
            ssl = slice(sc * SC, (sc + 1) * SC)
            for key, w_sb, b_sb, dst in (
                ("q", wq_sb, qb_sb, qt),
                ("k", wk_sb, kb_sb, kt_),
            ):
                if key not in which:
                    continue
                x_sb = x_sbs[key]
                for oc in ocs:
                    ps = pmix.tile([128, SC], F32, tag="mm")
                    for dk in range(DK):
                        nc.tensor.matmul(
                            ps[:],
                            lhsT=w_sb[:, dk, oc * 128 : (oc + 1) * 128],
                            rhs=x_sb[:, dk, :],
                            start=(dk == 0),
                            stop=(dk == DK - 1),
                        )
                    if zero_bias:
                        nc.vector.tensor_copy(
                            out=dst[0:64, 2 * oc, ssl], in_=ps[0:64, :]
                        )
                        nc.vector.tensor_copy(
                            out=dst[64:128, 2 * oc + 1, ssl], in_=ps[64:128, :]
                        )
                    else:
                        nc.vector.tensor_scalar(
                            out=dst[0:64, 2 * oc, ssl],
                            in0=ps[0:64, :],
                            scalar1=b_sb[0:64, oc : oc + 1],
                            scalar2=None,
                            op0=mybir.AluOpType.add,
                        )
                        nc.vector.tensor_scalar(
                            out=dst[64:128, 2 * oc + 1, ssl],
                            in0=ps[64:128, :],
                            scalar1=b_sb[64:128, oc : oc + 1],
                            scalar2=None,
                            op0=mybir.AluOpType.add,
                        )
                    yield
            if "v" not in which:
                return
            xv_sb = x_sbs["v"]
            for sth in range(SC // KT):
                st = sc * (SC // KT) + sth
                ps = pmix.tile([128, SC], F32, tag="mm")
                for dk in range(DK):
                    nc.tensor.matmul(
                        ps[:, :OG],
                        lhsT=xv_sb[:, dk, sth * KT : (sth + 1) * KT],
                        rhs=wv_sb[:, dk, :],
                        start=(dk == 0),
                        stop=(dk == DK - 1),
                    )
                nc.vector.tensor_copy(
                    out=vh1[:, st, :, 64:128],
                    in_=ps[:, :OG].rearrange("p (h d) -> p h d", d=DEPTH),
                )
                yield

        def attn_chain(h, j, want=2):
            """Generator: one head's attention for q-chunk j.  Yields at
            group boundaries; AV matmuls trail their group's scores by one
            group so the PE queue never blocks on the exp.  Paired chains
            run in lockstep, each owning one of the two score psum pools
            (want=2 -> 2-bank pool, want=3 -> 3-bank pool), so the pair's
            interleaved groups never contend for banks."""
            odd = h % 2
            pb = odd * 64
            ch = h // 2
            tiles = ktiles[j]
            first_t, last_t = tiles[0][0], tiles[-1][0]
            ps_av = pav.tile([128, QC], F32, tag="pav")

            def emit_av(grp, ex):
                for r, (t, lo, tri) in enumerate(grp):
                    nc.tensor.matmul(
                        ps_av[:, lo * 128 :],
                        lhsT=vh1[:, t, h, :],
                        rhs=ex[:, r, lo * 128 :],
                        start=(t == first_t),
                        stop=(t == last_t),
                    )

            pool, width, tg = (psc2, 2, "g2") if want == 2 else (psc3, 3, "g3")
            pending = None
            for grp, lm in _make_groups(tiles, want):
                ps_g = pool.tile([128, width, QC], F32, tag=tg)
                for r, (t, lo, tri) in enumerate(grp):
                    nc.tensor.matmul(
                        ps_g[:, r, lo * 128 :],
                        lhsT=kt_[:, h, t * KT : (t + 1) * KT],
                        rhs=qt[:, h, j * QC + lo * 128 : (j + 1) * QC],
                        start=True,
                        stop=True,
                    )
                ex = exps.tile([128, 3, QC], BF16, tag="exps")
                nc.scalar.activation(
                    out=ex[:, : len(grp), lm * 128 :],
                    in_=ps_g[:, : len(grp), lm * 128 :],
                    func=Exp,
                    scale=0.125,
                )
                for r, (t, lo, tri) in enumerate(grp):
                    for i, uid in tri:
                        nc.vector.tensor_mul(
                            ex[:, r, i * 128 : (i + 1) * 128],
                            ex[:, r, i * 128 : (i + 1) * 128],
                            mk_sb[:, uid, :],
                        )
                yield
                if pending is not None:
                    emit_av(*pending)
                    yield
                pending = (grp, ex)
            emit_av(*pending)
            # epilogue: the denominator sits replicated on psum partitions
            # 0-63; recip it in place (base partition 0 as the custom op
            # requires), then normalize.  Only the half that must cross
            # partitions takes a DMA hop.
            jc = slice(j * QC, (j + 1) * QC)
            rec_sb = small.tile([128, QC], F32, tag="rec_sb")
            nc.vector.reciprocal_approx_fast(rec_sb[0:64, :], ps_av[0:64, :])
            if odd:
                # av^T already sits on partitions 64-127; shift the recip up
                rec2 = bcastp.tile([128, QC], F32, tag="bc")
                nc.sync.dma_start(rec2[64:128, :], rec_sb[0:64, :])
                nc.vector.tensor_mul(
                    avb[64:128, ch, jc], ps_av[64:128, :], rec2[64:128, :]
                )
            else:
                # shift av^T down to partitions 0-63, normalize there
                tmp = bcastp.tile([128, QC], F32, tag="avtmp")
                nc.vector.tensor_copy(out=tmp[64:128, :], in_=ps_av[64:128, :])
                nc.sync.dma_start(avf[0:64, ch, jc], tmp[64:128, :])
                nc.vector.tensor_mul(
                    avb[0:64, ch, jc], avf[0:64, ch, jc], rec_sb[0:64, :]
                )
            yield

        def dense_pieces(jj, evac="v"):
            """Generator: dense output rows for q-chunk jj (4 stripes).
            evac picks the psum-evacuation engine ("s" for the kernel tail
            where ScalarE has run out of exps)."""
            for st in range(jj * 4, (jj + 1) * 4):
                ot = ost.tile([128, D_MODEL], BF16, tag="ostage")
                for oc in range(2):
                    ps = pmix.tile([128, SC], F32, tag="mm")
                    for co in range(2):
                        nc.tensor.matmul(
                            ps[:],
                            lhsT=avb[:, co, st * 128 : (st + 1) * 128],
                            rhs=wd_sb[:, co, oc * 512 : (oc + 1) * 512],
                            start=(co == 0),
                            stop=(co == 1),
                        )
                    if evac == "s":
                        nc.scalar.copy(
                            out=ot[:, oc * 512 : (oc + 1) * 512], in_=ps[:]
                        )
                    else:
                        nc.vector.tensor_copy(
                            out=ot[:, oc * 512 : (oc + 1) * 512], in_=ps[:]
                        )
                nc.sync.dma_start(outp[st * 128 : (st + 1) * 128, :], ot[:])
                yield

        # ---- interleaved emission: during attention of chunk sc, weave in
        # filler pieces (projections for later chunks, dense for earlier
        # ones) so the in-order PE queue always has exp-independent matmuls.
        # Dense work is deferred toward the last windows, which have no
        # projection left to hide the exp latency behind. ----
        def chain_gens(*gens):
            for g in gens:
                yield from g

        # q/k projections of chunk 0 run eagerly; its v pieces become the
        # first fillers inside attention(0)
        for _ in proj_pieces(0, x0, "qk"):
            pass
        nc.sync.dma_start(wd_sb[:], wd)  # dense-weight prefetch

        att_yields = [
            4 * (len(_make_groups(ktiles[sc], 2)) + len(_make_groups(ktiles[sc], 3)))
            for sc in range(NSC)
        ]
        fillers = {
            0: (lambda x1: [proj_pieces(0, x0, "v"), proj_pieces(1, x1)], 12),
            1: (lambda x2: [proj_pieces(2, x2)], 8),
            2: (lambda x3: [proj_pieces(3, x3), dense_pieces(0)], 12),
            3: (lambda _: [dense_pieces(1)], 4),
        }
        for sc in range(NSC):
            # issue next chunk's loads up front so they overlap this window
            xn = proj_load(sc + 1) if sc + 1 < NSC else None
            gens, n_other = fillers[sc]
            other = chain_gens(*gens(xn))
            ratio = max(1, att_yields[sc] // (n_other + 1))
            cnt = 0
            done = False
            for h0 in (0, 2):
                # run the pair's chains in lockstep, one step apiece
                pair = [attn_chain(h0, sc, 2), attn_chain(h0 + 1, sc, 3)]
                while pair:
                    for g in list(pair):
                        try:
                            next(g)
                        except StopIteration:
                            pair.remove(g)
                            continue
                        cnt += 1
                        if cnt % ratio == 0 and not done:
                            try:
                                next(other)
                            except StopIteration:
                                done = True
            for _ in other:
                pass
        # tail: the deferred dense stripes keep the PE busy while the last
        # chains' epilogues drain; ScalarE (out of exps by now) evacuates
        for _ in chain_gens(dense_pieces(2, evac="s"), dense_pieces(3, evac="s")):
            pass

    nc.compile()
    return nc


def _classify_mask(mask):
    """Classify 128(k) x 128(q) score blocks from the actual mask contents.

    Returns (ktiles, mk_arr):
      ktiles[j]: list of (t, lo, tri) per computed k-tile for q-chunk j:
        lo: first kept 128-col block within the 512-wide q-chunk (cols
            [0, lo*128) are fully masked and simply never computed/read)
        tri: [(col_block, uid), ...] 128-col blocks needing a factor mult
      mk_arr: [128, NU, 128] bf16 multiplicative factors exp(-1e9*m/8)
    """
    m2 = np.asarray(mask, dtype=np.float32).reshape(S, S)
    F = np.exp(m2 * np.float32(-1.25e8))  # exp(-1e9*m/8); 0/1 masks -> 0/1
    if (F.max(axis=1) == 0.0).any():
        raise RuntimeError("mask has fully-masked rows; unsupported")
    blocks = F.reshape(NKT, 128, NKT, 128)  # [qi, qr, t, kr]
    kept = (blocks == 1.0).all(axis=(1, 3))  # [qi, t]
    skip = (blocks == 0.0).all(axis=(1, 3))

    NB = QC // 128  # 128-col blocks per q-chunk
    ktiles = []
    uniq = {}
    mk_tiles = []

    def factor_uid(qi, t):
        fb = np.ascontiguousarray(
            F[qi * 128 : (qi + 1) * 128, t * KT : (t + 1) * KT].T
        ).astype(NPBF16)
        key = fb.tobytes()
        if key not in uniq:
            uniq[key] = len(mk_tiles)
            mk_tiles.append(fb)
        return uniq[key]

    for j in range(NQC):
        qis = list(range(j * NB, (j + 1) * NB))
        tl = []
        for t in range(NKT):
            stats = [
                "k" if kept[qi, t] else ("s" if skip[qi, t] else "m")
                for qi in qis
            ]
            if all(s == "s" for s in stats):
                continue
            lo = next(i for i, s in enumerate(stats) if s != "s")
            tri = []
            for i in range(lo, NB):
                if stats[i] == "k":
                    continue
                # mixed OR interior skip (multiply by its factor / zeros)
                tri.append((i, factor_uid(qis[i], t)))
            tl.append((t, lo, tri))
        if not tl:
            raise RuntimeError("q-chunk with no kept k-tiles; unsupported")
        # the first computed tile must span the full chunk (av 'start' MM)
        if tl[0][1] != 0:
            t0, _, tri0 = tl[0]
            tri0 = [(i, u) for i, u in tri0]
            have = {i for i, _ in tri0}
            for i in range(tl[0][1]):
                if i not in have:
                    tri0.append((i, factor_uid(qis[i], t0)))
            tl[0] = (t0, 0, sorted(tri0))
        ktiles.append(tl)
    if not mk_tiles:
        mk_tiles.append(np.ones((128, KT), dtype=NPBF16))
    mk_arr = np.ascontiguousarray(np.stack(mk_tiles, axis=0).transpose(1, 0, 2))
    return ktiles, mk_arr


def _xt_prep(x):
    """[S, D] f32 -> [NSC, 128, DK, SC] bf16, d-major, contiguous S-quarters."""
    xt = x.T.astype(NPBF16)  # [D, S]
    a = xt.reshape(DK, 128, NSC, SC).transpose(2, 1, 0, 3)
    return np.ascontiguousarray(a)


def kernel(v, k, q, mask, wq_w, wq_b, wk_w, wk_b, wv_w, wv_b, dense_w, dense_b):
    global LAST_RESULT
    v = np.asarray(v, dtype=np.float32)
    k = np.asarray(k, dtype=np.float32)
    q = np.asarray(q, dtype=np.float32)
    mask = np.asarray(mask, dtype=np.float32)
    wq_w = np.asarray(wq_w, dtype=np.float32)
    wk_w = np.asarray(wk_w, dtype=np.float32)
    wv_w = np.asarray(wv_w, dtype=np.float32)
    dense_w = np.asarray(dense_w, dtype=np.float32)
    wq_b = np.asarray(wq_b, dtype=np.float32)
    wk_b = np.asarray(wk_b, dtype=np.float32)
    wv_b = np.asarray(wv_b, dtype=np.float32)
    dense_b = np.asarray(dense_b, dtype=np.float32)

    ktiles, mk_arr = _classify_mask(mask)
    zero_bias = not (np.any(wq_b) or np.any(wk_b))
    key = (
        tuple(tuple((t, lo, tuple(tri)) for t, lo, tri in tl) for tl in ktiles),
        mk_arr.shape[1],
        zero_bias,
    )
    if key not in _CACHE:
        _CACHE[key] = _build(ktiles, mk_arr.shape[1], zero_bias)
    nc = _CACHE[key]

    # per-batch inputs (shared by the 4 cores of each batch)
    xq_b = [_xt_prep(q[b]) for b in range(B)]
    xk_b = [_xt_prep(k[b]) for b in range(B)]
    xv_b = [_xt_prep(v[b]) for b in range(B)]

    # per-group weights
    def wslice(w, g):
        ws = w[g * OG : (g + 1) * OG, :].T.astype(NPBF16)  # [D, OG]
        return np.ascontiguousarray(ws.reshape(DK, 128, OG).transpose(1, 0, 2))

    def bslice(b_, g):
        return np.ascontiguousarray(
            b_[g * OG : (g + 1) * OG].astype(np.float32).reshape(2, 128).T
        )

    wq_g = [wslice(wq_w, g) for g in range(GROUPS)]
    wk_g = [wslice(wk_w, g) for g in range(GROUPS)]
    wv_g = [wslice(wv_w, g) for g in range(GROUPS)]
    qb_g = [bslice(wq_b, g) for g in range(GROUPS)]
    kb_g = [bslice(wk_b, g) for g in range(GROUPS)]
    wd_g = []
    for g in range(GROUPS):
        ds = dense_w[:, g * OG : (g + 1) * OG].T.astype(NPBF16)  # [OG, D]
        wd_g.append(np.ascontiguousarray(ds.reshape(2, 128, D_MODEL).transpose(1, 0, 2)))

    in_maps = []
    for c in range(N_CORES):
        b, g = c // GROUPS, c % GROUPS
        in_maps.append(
            {
                "xq": xq_b[b],
                "xk": xk_b[b],
                "xv": xv_b[b],
                "wq": wq_g[g],
                "wk": wk_g[g],
                "wv": wv_g[g],
                "wd": wd_g[g],
                "qb": qb_g[g],
                "kb": kb_g[g],
                "mk": mk_arr,
            }
        )

    kw = dict(trace=True, **TRACE_KW) if TRACE else {}
    res = run_bass_kernel_spmd(nc, in_maps, core_ids=list(range(N_CORES)), **kw)
    LAST_RESULT = res

    corr = dense_w @ wv_b + dense_b  # v-bias pushed through dense, + dense bias
    out = np.empty((B, S, D_MODEL), dtype=np.float32)
    for b in range(B):
        acc = np.zeros((S, D_MODEL), dtype=np.float32)
        for g in range(GROUPS):
            acc += np.asarray(res.results[b * GROUPS + g]["outp"], dtype=np.float32)
        out[b] = acc + corr
    return out
